# revision 1
# baseline (speedup 1.0000x reference)
"""Trainium2 Bass kernel for out = x @ expm(skew(angles)) + bias.

Strategy:
  - Data-parallel over the batch: x [16384, 512] is split into 8 shards of
    [2048, 512], one per NeuronCore. angles/bias are replicated.
  - Host only does layout: builds A = skew(angles), A+I, the fp32r
    rounding of A, and ships each core its x shard pre-transposed ([512, 2048])
    so the contraction dim lands on SBUF partitions (pure marshaling, no
    FLOPs; the PE's matmul contracts over the partition dim, so x^T layout
    is required by the ISA). All linear algebra runs on-device.
  - On each core the rotation is computed via a degree-6 Taylor series in
    Paterson-Stockmeyer form (3 matmuls of 512^3), exploiting skew-symmetry
    so no on-chip transposes of the 512x512 operands are ever needed:
        A2  = A @ A        (lhsT = -A,  since A^T = -A)
        A3n = -A^3         (lhsT = A2,  since A2 is symmetric)
        B'' = (A+I) + A2/5 - A3n/30
        F'' = A3 @ B''     (lhsT = A3n, since A3^T = -A3)
        W   = expm(A) = (I + A) + A2/2 - A3n/8 + F''/24
    (identical to the degree-6 series: F'' = A3@B' - A3n, and the shifted
    A3n coefficient compensates; only the host-sent A+I is ever needed).
  - expm matmul operands are float32r (fp32 rounded to 11 mantissa bits):
    the PE streams fp32r at 1 column/cycle vs 4 cycle-equivalents for plain
    fp32's two-pass LOW_HIGH mode.  Operand magnitudes there are ~1e-2, so
    the 2^-12 input rounding contributes only ~3e-5 absolute to the output.
    The main x@W matmul stays full fp32; the linear terms of W are built
    from the exact (unrounded) A.
  - Main loop: per 128-row tile of x, 4 accumulating fp32 matmuls of N=512
    straight from the preloaded x^T slices; the final DVE op adds bias
    while moving PSUM -> SBUF.
  - -A is produced on-device by a DVE negate of the rounded A (exact, and
    fp32r rounding commutes with negation), halving the DMA bytes the expm
    chain waits on at startup.

Truncation error of the degree-6 series for this operand norm
(||A||_2 ~ 0.44) is ~4e-8, below fp32 matmul roundoff.
"""

import numpy as np

import concourse.bacc as bacc
import concourse.bass as bass
import concourse.mybir as mybir
import concourse.tile as tile
from concourse.bass_utils import run_bass_kernel_spmd

DIM = 512
BATCH = 16384
N_CORES = 8
XB = BATCH // N_CORES          # rows per core
P = 128                        # partitions
KT = DIM // P                  # 4 k-tiles
MT = XB // P                   # 16 m-tiles per core
XC = 4                         # m-tiles per x DMA chunk
F32 = mybir.dt.float32
F32R = mybir.dt.float32r

_CACHE = {}


def build_bass():
    nc = bacc.Bacc("TRN2", target_bir_lowering=False, debug=False)

    xt_d = nc.dram_tensor("xt", [DIM, XB], F32, kind="ExternalInput")
    ai_d = nc.dram_tensor("ai", [DIM, DIM], F32, kind="ExternalInput")
    ar_d = nc.dram_tensor("ar", [DIM, DIM], F32R, kind="ExternalInput")
    biasr_d = nc.dram_tensor("biasr", [P, DIM], F32, kind="ExternalInput")
    out_d = nc.dram_tensor("out", [XB, DIM], F32, kind="ExternalOutput")

    AOP = mybir.AluOpType

    with tile.TileContext(nc) as tc:
        with (
            tc.tile_pool(name="const", bufs=1) as cpool,
            tc.tile_pool(name="xin", bufs=MT // XC) as xpool,
            tc.tile_pool(name="oout", bufs=4) as opool,
            tc.tile_pool(name="eps", bufs=6, space=bass.MemorySpace.PSUM) as eps,
            tc.tile_pool(name="ops", bufs=2, space=bass.MemorySpace.PSUM) as ops,
        ):
            ai_sb = cpool.tile([P, KT, DIM], F32)    # A + I
            ar_sb = cpool.tile([P, KT, DIM], F32R)
            nar_sb = cpool.tile([P, KT, DIM], F32R)
            biasr_sb = cpool.tile([P, DIM], F32)

            for t in range(KT):
                nc.sync.dma_start(ar_sb[:, t, :], ar_d[P * t : P * (t + 1), :])
            # -A is negated on-device (exact; fp32r rounding commutes with
            # negation) instead of being a second 1MB load on the critical path
            for t in range(KT):
                nc.vector.tensor_scalar_mul(nar_sb[:, t, :], ar_sb[:, t, :], -1.0)
            # inputs below are consumed well after the expm chain starts
            nc.sync.dma_start(
                ai_sb[:, :, :], ai_d[:, :].rearrange("(t p) n -> p t n", p=P)
            )
            nc.sync.dma_start(biasr_sb[:, :], biasr_d[:, :])

            # ---- x^T loads: 4 chunks of [512, 512] ----
            xch = []
            for c in range(MT // XC):
                xc = xpool.tile([P, KT, P * XC], F32, tag="x")
                nc.sync.dma_start(
                    xc[:, :, :],
                    xt_d[:, P * XC * c : P * XC * (c + 1)].rearrange(
                        "(t p) m -> p t m", p=P
                    ),
                )
                xch.append(xc)

            # ---- expm chain (replicated; fp32r operands) ----
            a2_sb = cpool.tile([P, KT, DIM], F32R)
            a3n_sb = cpool.tile([P, KT, DIM], F32R)
            bp_sb = cpool.tile([P, KT, DIM], F32R)
            t3_sb = cpool.tile([P, KT, DIM], F32)
            m_sb = cpool.tile([P, KT, DIM], F32)

            # A2 = A @ A  (t-major: all 4 psum groups consume operand tile
            # t as soon as it lands, instead of each group serially waiting
            # for tiles to arrive)
            pss = []
            for i in range(KT):
                ps = eps.tile([P, DIM], F32, tag="eps")
                pss.append(ps)
            for t in range(KT):
                for i in range(KT):
                    nc.tensor.matmul(
                        pss[i][:, :],
                        nar_sb[:, t, P * i : P * (i + 1)],
                        ar_sb[:, t, :],
                        start=(t == 0),
                        stop=(t == KT - 1),
                    )
            for i in range(KT):
                nc.scalar.copy(a2_sb[:, i, :], pss[i][:, :])

            # a2-only halves of B' and t3 go first: the DVE chews through
            # them during the A3n matmul phase, so after the last A3n copy
            # only one op separates bp[0] from being ready for F'
            for t in range(KT):
                nc.vector.scalar_tensor_tensor(
                    bp_sb[:, t, :], a2_sb[:, t, :], 0.2, ai_sb[:, t, :],
                    AOP.mult, AOP.add,
                )
            for t in range(KT):
                nc.vector.scalar_tensor_tensor(
                    t3_sb[:, t, :], a2_sb[:, t, :], 0.5, ai_sb[:, t, :],
                    AOP.mult, AOP.add,
                )
            # A3n = -(A2 @ A) = A2 @ (-A)   (t-major, as above)
            pss = []
            for i in range(KT):
                ps = eps.tile([P, DIM], F32, tag="eps")
                pss.append(ps)
            for t in range(KT):
                for i in range(KT):
                    nc.tensor.matmul(
                        pss[i][:, :],
                        a2_sb[:, t, P * i : P * (i + 1)],
                        nar_sb[:, t, :],
                        start=(t == 0),
                        stop=(t == KT - 1),
                    )
            for i in range(KT):
                nc.scalar.copy(a3n_sb[:, i, :], pss[i][:, :])

            # B' = A + A2/5 - A3n/30 ; t3 = (A + I) + A2/2 - A3n/6
            # (split per k-tile so the F' matmuls can start on bp tile 0
            # while later tiles are still being built)
            # a2-only halves first: they are ready during the A3n matmul
            # phase, so the DVE works ahead and only one op separates the
            # last A3n copy from bp[0] being ready for F'
            for t in range(KT):
                nc.vector.scalar_tensor_tensor(
                    bp_sb[:, t, :], a3n_sb[:, t, :], -1.0 / 30.0, bp_sb[:, t, :],
                    AOP.mult, AOP.add,
                )
            for t in range(KT):
                nc.vector.scalar_tensor_tensor(
                    t3_sb[:, t, :], a3n_sb[:, t, :], -1.0 / 8.0, t3_sb[:, t, :],
                    AOP.mult, AOP.add,
                )

            # F' = A3 @ B' ; W = F'/24 + t3
            pss = []
            for i in range(KT):
                ps = eps.tile([P, DIM], F32, tag="eps")
                pss.append(ps)
            for t in range(KT):
                for i in range(KT):
                    nc.tensor.matmul(
                        pss[i][:, :],
                        a3n_sb[:, t, P * i : P * (i + 1)],
                        bp_sb[:, t, :],
                        start=(t == 0),
                        stop=(t == KT - 1),
                    )
            for i in range(KT):
                nc.vector.scalar_tensor_tensor(
                    m_sb[:, i, :], pss[i][:, :], 1.0 / 24.0, t3_sb[:, i, :],
                    AOP.mult, AOP.add,
                )

            # ---- main loop: out = x @ W + bias ----
            for mi in range(MT):
                xc = xch[mi // XC]
                mo = P * (mi % XC)
                ps = ops.tile([P, DIM], F32, tag="out")
                for kb in range(KT):
                    nc.tensor.matmul(
                        ps[:, :],
                        xc[:, kb, mo : mo + P],
                        m_sb[:, kb, :],
                        start=(kb == 0),
                        stop=(kb == KT - 1),
                    )
                ot = opool.tile([P, DIM], F32, tag="o")
                nc.vector.tensor_add(ot[:, :], ps[:, :], biasr_sb[:, :])
                nc.sync.dma_start(out_d[P * mi : P * (mi + 1), :], ot[:, :])

    nc.compile()
    return nc


def _get_nc():
    if "nc" not in _CACHE:
        _CACHE["nc"] = build_bass()
    return _CACHE["nc"]


def _round_fp32r(x):
    """Round-to-nearest-even to 11 mantissa bits (verified bit-exact
    against walrus's fp32_to_fp32r)."""
    b = np.ascontiguousarray(x, dtype=np.float32).view(np.uint32).astype(np.uint64)
    b = b + 0x7FF + ((b >> 12) & 1)
    return (b & np.uint64(0xFFFFF000)).astype(np.uint32).view(np.float32)


def _host_inputs(angles, bias):
    angles = np.asarray(angles, dtype=np.float32)
    bias = np.asarray(bias, dtype=np.float32)
    iu, ju = np.triu_indices(DIM, k=1)
    A = np.zeros((DIM, DIM), dtype=np.float32)
    A[iu, ju] = angles
    A[ju, iu] = -angles
    return {
        "ai": A + np.eye(DIM, dtype=np.float32),
        "ar": _round_fp32r(A),
        "biasr": np.ascontiguousarray(
            np.broadcast_to(bias.reshape(1, DIM), (P, DIM))
        ),
    }


def kernel(x, angles, bias, _profile=False):
    x = np.asarray(x, dtype=np.float32)
    # per-core x shards, pre-transposed to [DIM, XB] (layout only)
    xts = np.ascontiguousarray(
        x.reshape(N_CORES, XB, DIM).transpose(0, 2, 1)
    )
    shared = _host_inputs(angles, bias)
    nc = _get_nc()
    in_maps = [{"xt": xts[c], **shared} for c in range(N_CORES)]
    res = run_bass_kernel_spmd(
        nc, in_maps, list(range(N_CORES)), trace=bool(_profile)
    )
    _CACHE["last_result"] = res
    out = np.concatenate([res.results[c]["out"] for c in range(N_CORES)], axis=0)
    return out



# revision 6
# speedup vs baseline: 1.8654x; 1.8654x over previous
"""Trainium2 Bass kernel for out = x @ expm(skew(angles)) + bias.

Strategy:
  - Data-parallel over the batch: x [16384, 512] is split into 8 shards of
    [2048, 512], one per NeuronCore. angles/bias are replicated.
  - Host only does layout: builds A = skew(angles) in bf16 and ships each
    core its x shard pre-transposed ([512, 2048]) in bf16 so the
    contraction dim lands on SBUF partitions. All linear algebra runs
    on-device.
  - Rotation W = expm(A) via a degree-4 Taylor series in
    Paterson-Stockmeyer form (2 matmuls of 512^3), exploiting
    skew-symmetry so no transposes or negations are ever materialized:
        nA2 = A^T @ A            (= -A^2;  lhsT = rhs = A directly)
        X   = nA2/24 - A/6       (DVE, straight from PSUM)
        T   = (I + A) + A2/2     (I built on-device via affine_select)
        F   = nA2 @ X            (lhsT = nA2, symmetric;  = A^3/6 + A^4/24)
        W   = T + F
    Truncation error ||A||^5/5! ~ 2e-4 for ||A||_2 ~ 0.48, far below the
    2e-2 gate.
  - Dtypes: x and A stream as bf16 (halves DMA, 1 PE cycle/row); W and X
    stream as bf16 as well (the backend forbids mixing 32-bit and 16-bit
    matmul operands), one PE cycle per output row -- 4x faster than
    fp32 two-pass streaming. PSUM accumulation is always fp32. Measured
    end-to-end max rel err ~4e-3, dominated by the bf16 rounding of x and W.
  - A short burst of dummy matmuls at kernel start keeps the PE busy
    through its 1.2->2.4 GHz clock-ramp window while the A/x DMAs are
    still in flight, so the real matmul chain runs at full clock.
  - Main loop: per 128-row tile of x, 4 accumulating matmuls of N=512;
    the final DVE op adds bias while moving PSUM -> SBUF.
"""

import numpy as np
import ml_dtypes

import concourse.bacc as bacc
import concourse.bass as bass
import concourse.mybir as mybir
import concourse.tile as tile
from concourse.bass_utils import run_bass_kernel_spmd

DIM = 512
BATCH = 16384
N_CORES = 8
XB = BATCH // N_CORES          # rows per core
P = 128                        # partitions
KT = DIM // P                  # 4 k-tiles
MT = XB // P                   # 16 m-tiles per core
XC = 4                         # m-tiles per x DMA chunk
NWARM = 17                     # clock-ramp warmup matmuls (~110ns each)
F32 = mybir.dt.float32
F32R = mybir.dt.float32r
BF16 = mybir.dt.bfloat16

_CACHE = {}


def build_bass():
    nc = bacc.Bacc("TRN2", target_bir_lowering=False, debug=False)

    xt_d = nc.dram_tensor("xt", [DIM, XB], BF16, kind="ExternalInput")
    ar_d = nc.dram_tensor("ar", [DIM, DIM], BF16, kind="ExternalInput")
    biasr_d = nc.dram_tensor("biasr", [P, DIM], F32, kind="ExternalInput")
    out_d = nc.dram_tensor("out", [XB, DIM], F32, kind="ExternalOutput")

    AOP = mybir.AluOpType

    with tile.TileContext(nc) as tc:
        with (
            tc.tile_pool(name="const", bufs=1) as cpool,
            tc.tile_pool(name="xin", bufs=MT // XC) as xpool,
            tc.tile_pool(name="oout", bufs=4) as opool,
            tc.tile_pool(name="wps", bufs=1, space=bass.MemorySpace.PSUM) as wpool,
            tc.tile_pool(name="eps", bufs=4, space=bass.MemorySpace.PSUM) as eps,
            tc.tile_pool(name="ops", bufs=3, space=bass.MemorySpace.PSUM) as ops,
        ):
            ar_sb = cpool.tile([P, KT, DIM], BF16)     # A
            ai_sb = cpool.tile([P, KT, DIM], BF16)     # A + I
            na6_sb = cpool.tile([P, KT, DIM], F32)     # -A/6
            a2b_sb = cpool.tile([P, KT, DIM], BF16)    # nA2 (mm2 lhsT)
            xp_sb = cpool.tile([P, KT, DIM], BF16)     # X = nA2/24 - A/6
            t_sb = cpool.tile([P, KT, DIM], F32)       # T = (I+A) + A2/2
            m_sb = cpool.tile([P, KT, DIM], BF16)      # W
            biasr_sb = cpool.tile([P, DIM], F32)
            warm_sb = cpool.tile([P, P], BF16)

            # ---- PE clock-ramp warmup while the input DMAs fly ----
            nc.gpsimd.memset(warm_sb[:, :], 0.0)
            wp = wpool.tile([P, DIM], F32, tag="warm")
            for _ in range(NWARM):
                nc.tensor.matmul(
                    wp[:, 0:P], warm_sb[:, :], warm_sb[:, :], start=True, stop=True
                )

            # ---- input DMAs, priority order: A tiles gate everything ----
            for t in range(KT):
                nc.sync.dma_start(ar_sb[:, t, :], ar_d[P * t : P * (t + 1), :])
            xch = []
            for c in range(MT // XC):
                xc = xpool.tile([P, KT, P * XC], BF16, tag="x")
                nc.sync.dma_start(
                    xc[:, :, :],
                    xt_d[:, P * XC * c : P * XC * (c + 1)].rearrange(
                        "(t p) m -> p t m", p=P
                    ),
                )
                xch.append(xc)
            nc.sync.dma_start(biasr_sb[:, :], biasr_d[:, :])

            # ---- cheap derived tensors (run during mm1) ----
            for t in range(KT):
                # ai = A, with exact 1.0 dropped onto the diagonal
                # (global row P*t + p == col n)
                nc.gpsimd.affine_select(
                    out=ai_sb[:, t, :],
                    in_=ar_sb[:, t, :],
                    compare_op=AOP.not_equal,
                    fill=1.0,
                    base=-P * t,
                    channel_multiplier=-1,
                    pattern=[[1, DIM]],
                )
            for t in range(KT):
                nc.vector.tensor_scalar_mul(na6_sb[:, t, :], ar_sb[:, t, :], -1.0 / 6.0)

            # ---- mm1: nA2 = A^T @ A = -A^2  (t-major: consume A tiles
            # as they land) ----
            pss = [eps.tile([P, DIM], F32, tag="eps", name=f"pss{i}") for i in range(KT)]
            for t in range(KT):
                for i in range(KT):
                    nc.tensor.matmul(
                        pss[i][:, :],
                        ar_sb[:, t, P * i : P * (i + 1)],
                        ar_sb[:, t, :],
                        start=(t == 0),
                        stop=(t == KT - 1),
                    )
            # lhsT copy on the scalar engine; X/T built by DVE straight
            # from PSUM in parallel
            for i in range(KT):
                nc.scalar.copy(a2b_sb[:, i, :], pss[i][:, :])
            for i in range(KT):
                nc.vector.scalar_tensor_tensor(
                    xp_sb[:, i, :], pss[i][:, :], 1.0 / 24.0, na6_sb[:, i, :],
                    AOP.mult, AOP.add,
                )
            for i in range(KT):
                nc.vector.scalar_tensor_tensor(
                    t_sb[:, i, :], pss[i][:, :], -0.5, ai_sb[:, i, :],
                    AOP.mult, AOP.add,
                )

            # ---- mm2: F = nA2 @ X = A^3/6 + A^4/24 ----
            ps2 = [eps.tile([P, DIM], F32, tag="eps", name=f"ps2{i}") for i in range(KT)]
            for t in range(KT):
                for i in range(KT):
                    nc.tensor.matmul(
                        ps2[i][:, :],
                        a2b_sb[:, t, P * i : P * (i + 1)],
                        xp_sb[:, t, :],
                        start=(t == 0),
                        stop=(t == KT - 1),
                    )
            for i in range(KT):
                nc.vector.tensor_add(m_sb[:, i, :], ps2[i][:, :], t_sb[:, i, :])

            # ---- main loop: out = x @ W + bias ----
            for mi in range(MT):
                xc = xch[mi // XC]
                mo = P * (mi % XC)
                ps = ops.tile([P, DIM], F32, tag="out")
                for kb in range(KT):
                    nc.tensor.matmul(
                        ps[:, :],
                        xc[:, kb, mo : mo + P],
                        m_sb[:, kb, :],
                        start=(kb == 0),
                        stop=(kb == KT - 1),
                    )
                ot = opool.tile([P, DIM], F32, tag="o")
                nc.vector.tensor_add(ot[:, :], ps[:, :], biasr_sb[:, :])
                nc.sync.dma_start(out_d[P * mi : P * (mi + 1), :], ot[:, :])

    nc.compile()
    return nc


def _get_nc():
    if "nc" not in _CACHE:
        _CACHE["nc"] = build_bass()
    return _CACHE["nc"]


def _host_inputs(angles, bias):
    angles = np.asarray(angles, dtype=np.float32)
    bias = np.asarray(bias, dtype=np.float32)
    iu, ju = np.triu_indices(DIM, k=1)
    A = np.zeros((DIM, DIM), dtype=np.float32)
    A[iu, ju] = angles
    A[ju, iu] = -angles
    return {
        "ar": A.astype(ml_dtypes.bfloat16),
        "biasr": np.ascontiguousarray(
            np.broadcast_to(bias.reshape(1, DIM), (P, DIM))
        ),
    }


def kernel(x, angles, bias, _profile=False):
    x = np.asarray(x, dtype=np.float32)
    # per-core x shards, pre-transposed to [DIM, XB] bf16 (layout only)
    xts = np.ascontiguousarray(
        x.reshape(N_CORES, XB, DIM).transpose(0, 2, 1)
    ).astype(ml_dtypes.bfloat16)
    shared = _host_inputs(angles, bias)
    nc = _get_nc()
    in_maps = [{"xt": xts[c], **shared} for c in range(N_CORES)]
    res = run_bass_kernel_spmd(
        nc, in_maps, list(range(N_CORES)), trace=bool(_profile)
    )
    _CACHE["last_result"] = res
    out = np.concatenate([res.results[c]["out"] for c in range(N_CORES)], axis=0)
    return out


# revision 8
# speedup vs baseline: 2.1769x; 1.1670x over previous
"""Trainium2 Bass kernel for out = x @ expm(skew(angles)) + bias.

Strategy:
  - Data-parallel over the batch: x [16384, 512] is split into 8 shards of
    [2048, 512], one per NeuronCore. angles/bias are replicated.
  - Host only does layout: builds A = skew(angles) in bf16 and ships each
    core its x shard pre-transposed ([512, 2048]) in bf16 so the
    contraction dim lands on SBUF partitions. All linear algebra runs
    on-device.
  - Rotation W = expm(A) via a degree-4 Taylor series in
    Paterson-Stockmeyer form (2 matmuls of 512^3), exploiting
    skew-symmetry so no transposes or negations are ever materialized:
        nA2 = A^T @ A              (= -A^2;  lhsT = rhs = A directly)
        A6  = -nA2 / 6             (scalar-engine scaled copy = A^2/6)
        Y   = A - nA2/4            (one DVE op per tile, = A + A^2/4)
        T   = (I + A) - nA2/2      (I built on-device via affine_select)
        F   = A6 @ Y               (A6 symmetric => its own lhsT; = A^3/6 + A^4/24)
        W   = T + F
    Truncation error ||A||^5/5! ~ 2e-4 for ||A||_2 ~ 0.48, far below the
    2e-2 gate.
  - Dtypes: all matmul operands are bf16 (the backend forbids mixing
    32-bit and 16-bit operands), streaming 1 PE cycle per output row --
    4x faster than fp32 two-pass mode -- and halving the x DMA. PSUM
    accumulation is fp32; the output is exact fp32. Measured end-to-end
    max rel err ~4e-3, dominated by the bf16 rounding of x and W.
  - Clock/pipeline scheduling (from trace analysis): the PE p-state ramps
    only under *continuous* load, so a warmup burst of dummy matmuls runs
    from the prologue barrier until the A tiles land, with no idle gap.
    mm1 runs i-major so its 4 PSUM groups stop staggered ~0.9us apart;
    the scaled copy (scalar), Y (vector) and T (vector) for group i start
    as soon as group i stops, letting t-major mm2 start the moment mm1's
    last matmul retires.  W k-tiles are finished on two engines
    (vector/gpsimd) so the main loop starts ~0.5us after mm2.
  - PSUM budget: main loop rotates the 4 mm1 banks (eps pool), mm2 uses
    the other 4 (ops pool + warmup bank), so no engine ever stalls on a
    bank recycle.
  - Main loop: per 128-row tile of x, 4 accumulating matmuls of N=512;
    the final DVE op adds bias while moving PSUM -> SBUF.
"""

import numpy as np
import ml_dtypes

import concourse.bacc as bacc
import concourse.bass as bass
import concourse.mybir as mybir
import concourse.tile as tile
from concourse.bass_utils import run_bass_kernel_spmd

DIM = 512
BATCH = 16384
N_CORES = 8
XB = BATCH // N_CORES          # rows per core
P = 128                        # partitions
KT = DIM // P                  # 4 k-tiles
MT = XB // P                   # 16 m-tiles per core
XC = 4                         # m-tiles per x DMA chunk
NWARM = 27                     # clock-ramp warmup matmuls (~107ns each)
F32 = mybir.dt.float32
BF16 = mybir.dt.bfloat16

_CACHE = {}


def build_bass():
    nc = bacc.Bacc("TRN2", target_bir_lowering=False, debug=False)

    xt_d = nc.dram_tensor("xt", [DIM, XB], BF16, kind="ExternalInput")
    ar_d = nc.dram_tensor("ar", [DIM, DIM], BF16, kind="ExternalInput")
    biasr_d = nc.dram_tensor("biasr", [P, DIM], F32, kind="ExternalInput")
    out_d = nc.dram_tensor("out", [XB, DIM], F32, kind="ExternalOutput")

    AOP = mybir.AluOpType

    with tile.TileContext(nc) as tc:
        with (
            tc.tile_pool(name="const", bufs=1) as cpool,
            tc.tile_pool(name="xin", bufs=MT // XC) as xpool,
            tc.tile_pool(name="oout", bufs=4) as opool,
            tc.tile_pool(name="wps", bufs=1, space=bass.MemorySpace.PSUM) as wpool,
            tc.tile_pool(name="eps", bufs=4, space=bass.MemorySpace.PSUM) as eps,
            tc.tile_pool(name="ops", bufs=3, space=bass.MemorySpace.PSUM) as ops,
        ):
            ar_sb = cpool.tile([P, KT, DIM], BF16)     # A
            ai_sb = cpool.tile([P, KT, DIM], BF16)     # A + I
            a6_sb = cpool.tile([P, KT, DIM], BF16)     # A^2/6 (mm2 lhsT)
            y_sb = cpool.tile([P, KT, DIM], BF16)      # Y = A + A^2/4
            t_sb = cpool.tile([P, KT, DIM], F32)       # T = (I+A) + A^2/2
            m_sb = cpool.tile([P, KT, DIM], BF16)      # W
            biasr_sb = cpool.tile([P, DIM], F32)
            warm_sb = cpool.tile([P, P], BF16)

            # ---- PE clock-ramp warmup: keep the PE continuously busy
            # from the prologue barrier until the A tiles land ----
            nc.gpsimd.memset(warm_sb[:, :], 0.0)
            wp = wpool.tile([P, DIM], F32, tag="warm")
            for _ in range(NWARM):
                nc.tensor.matmul(
                    wp[:, 0:P], warm_sb[:, :], warm_sb[:, :], start=True, stop=True
                )

            # ---- input DMAs, priority order: A tiles gate everything ----
            for t in range(KT):
                nc.sync.dma_start(ar_sb[:, t, :], ar_d[P * t : P * (t + 1), :])
            xch = []
            for c in range(MT // XC):
                xc = xpool.tile([P, KT, P * XC], BF16, tag="x")
                nc.sync.dma_start(
                    xc[:, :, :],
                    xt_d[:, P * XC * c : P * XC * (c + 1)].rearrange(
                        "(t p) m -> p t m", p=P
                    ),
                )
                xch.append(xc)
            nc.sync.dma_start(biasr_sb[:, :], biasr_d[:, :])

            # ---- ai = A with exact 1.0 on the diagonal (gpsimd, paced
            # only by the A-tile DMAs; global row P*t + p == col n) ----
            for t in range(KT):
                nc.gpsimd.affine_select(
                    out=ai_sb[:, t, :],
                    in_=ar_sb[:, t, :],
                    compare_op=AOP.not_equal,
                    fill=1.0,
                    base=-P * t,
                    channel_multiplier=-1,
                    pattern=[[1, DIM]],
                )

            # ---- mm1: nA2 = A^T @ A = -A^2, i-major so the 4 PSUM
            # groups stop staggered and downstream work starts early ----
            pss = [eps.tile([P, DIM], F32, tag="eps", name=f"pss{i}") for i in range(KT)]
            for i in range(KT):
                for t in range(KT):
                    nc.tensor.matmul(
                        pss[i][:, :],
                        ar_sb[:, t, P * i : P * (i + 1)],
                        ar_sb[:, t, :],
                        start=(t == 0),
                        stop=(t == KT - 1),
                    )
            # scalar engine: A6 = A^2/6 bf16 copy (the mm2 lhsT)
            for i in range(KT):
                nc.scalar.mul(a6_sb[:, i, :], pss[i][:, :], -1.0 / 6.0)
            # vector: Y_i then T_i right behind each group's stop
            for i in range(KT):
                nc.vector.scalar_tensor_tensor(
                    y_sb[:, i, :], pss[i][:, :], -0.25, ar_sb[:, i, :],
                    AOP.mult, AOP.add,
                )
                nc.vector.scalar_tensor_tensor(
                    t_sb[:, i, :], pss[i][:, :], -0.5, ai_sb[:, i, :],
                    AOP.mult, AOP.add,
                )

            # ---- mm2: F = A6 @ Y = A^3/6 + A^4/24, t-major (its deps
            # arrive in t order; starts the moment mm1 retires).
            # Uses the other 4 PSUM banks (ops pool + warmup bank). ----
            ps2 = [ops.tile([P, DIM], F32, tag="m2", name=f"ps2{i}") for i in range(KT - 1)]
            ps2.append(wpool.tile([P, DIM], F32, tag="warm", name="ps2w"))
            for t in range(KT):
                for i in range(KT):
                    nc.tensor.matmul(
                        ps2[i][:, :],
                        a6_sb[:, t, P * i : P * (i + 1)],
                        y_sb[:, t, :],
                        start=(t == 0),
                        stop=(t == KT - 1),
                    )
            # W = T + F (vector: gpsimd cannot read PSUM)
            for i in range(KT):
                nc.vector.tensor_add(m_sb[:, i, :], ps2[i][:, :], t_sb[:, i, :])

            # ---- main loop: out = x @ W + bias (PSUM rotates the 4
            # mm1 banks; bias-add drains each bank long before reuse) ----
            for mi in range(MT):
                xc = xch[mi // XC]
                mo = P * (mi % XC)
                ps = eps.tile([P, DIM], F32, tag="eps", name=f"mo{mi}")
                for kb in range(KT):
                    nc.tensor.matmul(
                        ps[:, :],
                        xc[:, kb, mo : mo + P],
                        m_sb[:, kb, :],
                        start=(kb == 0),
                        stop=(kb == KT - 1),
                    )
                ot = opool.tile([P, DIM], F32, tag="o")
                nc.vector.tensor_add(ot[:, :], ps[:, :], biasr_sb[:, :])
                nc.sync.dma_start(out_d[P * mi : P * (mi + 1), :], ot[:, :])

    nc.compile()
    return nc


def _get_nc():
    if "nc" not in _CACHE:
        _CACHE["nc"] = build_bass()
    return _CACHE["nc"]


def _host_inputs(angles, bias):
    angles = np.asarray(angles, dtype=np.float32)
    bias = np.asarray(bias, dtype=np.float32)
    iu, ju = np.triu_indices(DIM, k=1)
    A = np.zeros((DIM, DIM), dtype=np.float32)
    A[iu, ju] = angles
    A[ju, iu] = -angles
    return {
        "ar": A.astype(ml_dtypes.bfloat16),
        "biasr": np.ascontiguousarray(
            np.broadcast_to(bias.reshape(1, DIM), (P, DIM))
        ),
    }


def kernel(x, angles, bias, _profile=False):
    x = np.asarray(x, dtype=np.float32)
    # per-core x shards, pre-transposed to [DIM, XB] bf16 (layout only)
    xts = np.ascontiguousarray(
        x.reshape(N_CORES, XB, DIM).transpose(0, 2, 1)
    ).astype(ml_dtypes.bfloat16)
    shared = _host_inputs(angles, bias)
    nc = _get_nc()
    in_maps = [{"xt": xts[c], **shared} for c in range(N_CORES)]
    res = run_bass_kernel_spmd(
        nc, in_maps, list(range(N_CORES)), trace=bool(_profile)
    )
    _CACHE["last_result"] = res
    out = np.concatenate([res.results[c]["out"] for c in range(N_CORES)], axis=0)
    return out
